# revision 24
# baseline (speedup 1.0000x reference)
"""Trainium2 Bass kernel for nn_MemoryCell (causal linear attention memory cell).

Math: the reference's sequential scan
    mem += outer(k_t, v_t); zeta += k_t; y_t = (q_t @ mem) / (q_t . zeta)
is causal linear attention, computed as chunked attention over superchunks of
SC=512 steps:
    y = Q @ Mhat + tril(Q K^T) @ Vhat      (Vhat has an all-ones column
                                            which produces the normalizer)
    Mhat += K^T Vhat  per superchunk.

Sharding (8 cores, feature/tensor parallel): each core projects its 256-wide
slice of gated Q^T/K^T from full x, all-gathers them (bf16 wire), and computes
its 256-wide V/Mhat/y column slice locally. y is concatenated on the host.

v3 structure:
  - A = tril(K Q^T) is computed as per-core partials over the local 256
    features (no gather needed) and AllReduce-summed on the collective
    engine — instead of every core redoing the full-D contraction (8x
    redundant PE work in v1).
  - K is gathered pre-transposed (t-major) for the Mhat update: the local
    k slice is PE-transposed in phase 1 and the stacked [8*TH, DV] gather
    serves plain strided kn loads in phase 3 (v1/v2 used per-tile
    DMA_TRANSPOSE loads whose descriptor generation saturated ACT).
  - q (feature-major) and kT ride ONE AllGather per half (2 MB/rank).
  - phase 1 is split per half into [q/k projections + partial A -> fire
    AG+AR] then [V projections] so the collectives start ~70us in and
    overlap the remaining compute. Vhat stays resident in SBUF.
  - DMA queues separated: loads on sync (p1) / scalar (p3), stores on
    gpsimd SWDGE, y on sync — a v1 bounce write stuck behind phase-3
    loads delayed an AllGather by ~100us.

dtypes: x/W/q/k and A wire in bf16; all PSUM accumulation fp32.
"""

import os

import numpy as np

T, D = 4096, 2048
NCORE = 8
DV = D // NCORE          # 256: v-columns per core
DVE = DV + 2             # v-columns + ones column + pad (fp32r needs even N)
P = 128
KD = D // P              # 16 feature tiles
SC = 512                 # superchunk length (t)
NSC = T // SC            # 8
NBLK = SC // P           # 4 blocks per superchunk
NSCH = NSC // 2          # superchunks per half
TH = T // 2
A_OFF = [0, 512, 896, 1152]           # packed col offset of row j in an A tile
A_SPAN = [512, 384, 256, 128]         # row j covers t in [j*128, 512)
A_W = 1280                            # packed width per superchunk

_CACHE = {}


def _cs_factors():
    idx = np.arange(D // 2, dtype=np.float32)
    thetas = np.float32(10000.0) ** (np.float32(-2.0) * idx)
    pos = np.arange(T, dtype=np.float32)
    ang = pos[:, None] * thetas[None, :]
    cos = np.repeat(np.cos(ang), 2, axis=-1)
    sin = np.repeat(np.sin(ang), 2, axis=-1)
    return (cos + sin).astype(np.float32)


def _build_nc():
    import concourse.bacc as bacc
    import concourse.mybir as mybir
    import concourse.tile as tile
    from concourse.bass import ts
    from concourse.masks import make_identity, make_upper_triangular

    f32 = mybir.dt.float32
    bf16 = mybir.dt.bfloat16
    SIG = mybir.ActivationFunctionType.Sigmoid
    MUL = mybir.AluOpType.mult

    nc = bacc.Bacc(num_devices=NCORE)

    xT = nc.dram_tensor("xT", [D, T], bf16, kind="ExternalInput")
    wqT = nc.dram_tensor("wqT", [D, DV], bf16, kind="ExternalInput")
    wkT = nc.dram_tensor("wkT", [D, DV], bf16, kind="ExternalInput")
    wvT = nc.dram_tensor("wvT", [D, DV], bf16, kind="ExternalInput")
    csT = nc.dram_tensor("csT", [DV, T], f32, kind="ExternalInput")
    y_out = nc.dram_tensor("y", [T, DV], f32, kind="ExternalOutput")

    xTv = xT[:, :].rearrange("(k p) t -> p k t", p=P)     # [128, 16, T]
    wqv = wqT[:, :].rearrange("(k p) n -> p k n", p=P)    # [128, 16, 256]
    wkv = wkT[:, :].rearrange("(k p) n -> p k n", p=P)
    wvv = wvT[:, :].rearrange("(k p) n -> p k n", p=P)
    csv = csT[:, :].rearrange("(k p) t -> p k t", p=P)    # [128, 2, T]

    with tile.TileContext(nc) as tc:
        with (
            tc.tile_pool(name="const", bufs=1) as constp,
            tc.tile_pool(name="dram", bufs=1, space="DRAM") as dramp,
            tc.tile_pool(name="mhat", bufs=1) as mhatp,
            tc.tile_pool(name="vhat", bufs=1) as vhp,
        ):
            triu = constp.tile([P, P], f32)
            make_upper_triangular(nc, triu[:], val=1.0, diag=True)
            ident = constp.tile([P, P], bf16)
            make_identity(nc, ident[:])

            A_E = NSCH * A_W   # packed partial-A elems per partition per half
            NQK = 2 * DV * TH  # flat bf16 elems per rank: q block + kT block
            NQKA = NQK + P * A_E  # + partial-A block riding the same gather
            qk_bounce = [dramp.tile([NQKA], bf16, name=f"qk_bounce{h}") for h in range(2)]
            qk_gath = [
                dramp.tile([NCORE, NQKA], bf16, addr_space="Shared", name=f"qk_gath{h}")
                for h in range(2)
            ]

            # bounce views: q block [DV, TH] feature-major; kT block [TH, DV]
            qb_v = [
                b[0:DV * TH].rearrange("(k p t) -> p k t", p=P, t=TH) for b in qk_bounce
            ]  # [128, 2, TH]
            ktb_v = [
                b[DV * TH : NQK].rearrange("(c b p n) -> p c b n", c=NSCH, b=NBLK, p=P)
                for b in qk_bounce
            ]  # [128(t), chunk, blk, 256]
            ab_v = [
                b[NQK:NQKA].rearrange("(p c) -> p c", c=A_E) for b in qk_bounce
            ]  # [128, 5120] packed partial A for this half
            # gathered views
            qg_v = [
                g[:, 0:DV * TH].rearrange("m (k p t) -> p m k t", k=2, p=P)
                for g in qk_gath
            ]  # [128, 8, 2, TH] feature-major q; tile k_glob = m*2+k
            ktg_v = [
                g[:, DV * TH : NQK].rearrange("m (t n) -> t m n", n=DV)
                for g in qk_gath
            ]  # [TH(t), 8, 256] t-major k
            ga_v = [
                g[:, NQK:NQKA].rearrange("m (p c) -> p m c", p=P) for g in qk_gath
            ]  # [128, 8, 5120] per-core partial A, summed on-core (no AllReduce)

            mhat = mhatp.tile([P, KD, DVE], bf16)         # [128, 16, 258]
            nc.vector.memset(mhat[:], 0.0)
            vhat = vhp.tile([P, NSC * NBLK, DVE], bf16)   # [128, 32, 258]
            nc.vector.memset(vhat[:, :, DV:DVE], 1.0)
            wv_sb = constp.tile([P, KD, DV], bf16)
            nc.sync.dma_start(wv_sb[:], wvv)

            def all_gather(src, dst):
                nc.gpsimd.collective_compute(
                    "AllGather",
                    mybir.AluOpType.bypass,
                    replica_groups=[list(range(NCORE))],
                    ins=[src.opt()],
                    outs=[dst.opt()],
                )

            # ---------------- Phase 1 ----------------
            with (
                tc.tile_pool(name="w", bufs=1) as wp,
                tc.tile_pool(name="xin", bufs=5) as xp,
                tc.tile_pool(name="csp", bufs=2) as csp,
                tc.tile_pool(name="qk", bufs=3) as qkp,
                tc.tile_pool(name="ktsb", bufs=2) as ktp,
                tc.tile_pool(name="asb", bufs=2) as asbp,
                tc.tile_pool(name="pj_ps", bufs=3, space="PSUM") as pjps,
                tc.tile_pool(name="px_ps", bufs=2, space="PSUM") as pxps,
                tc.tile_pool(name="pa_ps", bufs=2, space="PSUM") as paps,
            ):
                wq_sb = wp.tile([P, KD, DV], bf16)
                nc.sync.dma_start(wq_sb[:], wqv)
                wk_sb = wp.tile([P, KD, DV], bf16)
                nc.sync.dma_start(wk_sb[:], wkv)

                xts = {}

                def seg_a(h):
                    """q/k projections+gates, kT transpose, partial A, bounce writes."""
                    for ch in range(NSCH):
                        c = h * NSCH + ch
                        xt = xp.tile([P, KD, SC], bf16, tag="xt", name=f"xt{c}")
                        nc.sync.dma_start(xt[:], xTv[:, :, ts(c, SC)])
                        xts[c] = xt
                        cst = csp.tile([P, 2, SC], f32, tag="cst")
                        nc.sync.dma_start(cst[:], csv[:, :, ts(c, SC)])

                        gs = {}
                        for nm, w_sb in (("q", wq_sb), ("k", wk_sb)):
                            g = qkp.tile([P, 2, SC], bf16, tag=f"g{nm}")
                            for do in range(2):
                                ps = pjps.tile([P, SC], f32, tag="pj")
                                for k in range(KD):
                                    nc.tensor.matmul(
                                        ps[:],
                                        w_sb[:, k, ts(do, P)],
                                        xt[:, k, :],
                                        start=(k == 0),
                                        stop=(k == KD - 1),
                                    )
                                nc.vector.tensor_mul(g[:, do, :], ps[:], cst[:, do, :])
                                nc.scalar.activation(
                                    g[:, do, :], g[:, do, :], SIG, scale=1.0 / D
                                )
                            gs[nm] = g
                        # q bounce (feature-major)
                        nc.scalar.dma_start(qb_v[h][:, :, ts(ch, SC)], gs["q"][:])
                        # kT via PE transpose -> bounce (t-major)
                        kt_sb = ktp.tile([P, NBLK, DV], bf16, tag="kt")
                        for tb in range(NBLK):
                            for do in range(2):
                                pst = pxps.tile([P, DV], bf16, tag="px")
                                nc.tensor.transpose(
                                    pst[:, 0:P], gs["k"][:, do, ts(tb, P)], ident[:]
                                )
                                nc.vector.tensor_copy(
                                    kt_sb[:, tb, ts(do, P)], pst[:, 0:P]
                                )
                        nc.scalar.dma_start(ktb_v[h][:, ch, :, :], kt_sb[:])

                        # partial A rows for this superchunk (local features)
                        for j in range(NBLK):
                            span = A_SPAN[j]
                            ps_a = paps.tile([P, SC], f32, tag="pa")
                            for kt in range(2):
                                nc.tensor.matmul(
                                    ps_a[:, 0:span],
                                    gs["k"][:, kt, ts(j, P)],
                                    gs["q"][:, kt, j * P : SC],
                                    start=(kt == 0),
                                    stop=(kt == 1),
                                )
                            a_sb = asbp.tile([P, SC], bf16, tag="asb")
                            nc.vector.tensor_mul(a_sb[:, 0:P], ps_a[:, 0:P], triu[:])
                            if span > P:
                                nc.vector.tensor_copy(a_sb[:, P:span], ps_a[:, P:span])
                            nc.scalar.dma_start(
                                ab_v[h][
                                    :, ch * A_W + A_OFF[j] : ch * A_W + A_OFF[j] + span
                                ],
                                a_sb[:, 0:span],
                            )

                def seg_b(h):
                    """V projections into resident vhat."""
                    for ch in range(NSCH):
                        c = h * NSCH + ch
                        xt = xts.pop(c)
                        for tt in range(NBLK):
                            psv = pxps.tile([P, DV], f32, tag="px")
                            for k in range(KD):
                                nc.tensor.matmul(
                                    psv[:],
                                    xt[:, k, ts(tt, P)],
                                    wv_sb[:, k, :],
                                    start=(k == 0),
                                    stop=(k == KD - 1),
                                )
                            nc.vector.tensor_copy(vhat[:, c * NBLK + tt, 0:DV], psv[:])

                seg_a(0)
                all_gather(qk_bounce[0], qk_gath[0])
                seg_b(0)
                seg_a(1)
                all_gather(qk_bounce[1], qk_gath[1])
                seg_b(1)

            # ---------------- Phase 3: chunked causal linear attention ----------------
            with (
                tc.tile_pool(name="asc", bufs=3) as ap_,
                tc.tile_pool(name="kn", bufs=3) as knp,
                tc.tile_pool(name="atsb", bufs=2) as atp,
                tc.tile_pool(name="gmp", bufs=3) as gmp,
                tc.tile_pool(name="ysb", bufs=6) as yp_,
                tc.tile_pool(name="rec", bufs=6) as recp,
                tc.tile_pool(name="y_ps", bufs=3, space="PSUM") as yps,
                tc.tile_pool(name="d_ps", bufs=3, space="PSUM") as dps,
            ):

                asums = {}

                def stage_front(s):
                    """Loads for superchunk s."""
                    h, sh = divmod(s, NSCH)
                    if sh == 0:
                        # sum the 8 gathered partial-A blocks on DVE
                        asum = atp.tile([P, A_E], bf16, tag="asum", name=f"asum{h}")
                        gm_prev = gmp.tile([P, A_E], bf16, tag="gm")
                        nc.sync.dma_start(gm_prev[:], ga_v[h][:, 0, :])
                        for m in range(1, NCORE):
                            gm = gmp.tile([P, A_E], bf16, tag="gm")
                            nc.sync.dma_start(gm[:], ga_v[h][:, m, :])
                            nc.vector.tensor_add(
                                asum[:], asum[:] if m > 1 else gm_prev[:], gm[:]
                            )
                        asums[h] = asum
                    a_sc = ap_.tile([P, NCORE, 2, SC], bf16, tag="a", name=f"a{s}")
                    for k2 in range(2):
                        nc.sync.dma_start(
                            a_sc[:, :, k2, :], qg_v[h][:, :, k2, ts(sh, SC)]
                        )
                    kn_s = knp.tile([P, NBLK, NCORE, DV], bf16, tag="kn", name=f"kn{s}")
                    for i in range(NBLK):
                        blk_h = sh * NBLK + i
                        nc.scalar.dma_start(
                            kn_s[:, i, :, :], ktg_v[h][ts(blk_h, P), :, :]
                        )
                    return (s, a_sc, kn_s, asums[h], sh * A_W)

                def stage_back(st):
                    """y blocks + Mhat update for a prepared superchunk."""
                    s, a_sc, kn_s, at_s, ab = st
                    for i in range(NBLK):
                        blk = s * NBLK + i
                        ps_y = yps.tile([P, DVE], f32, tag="y")
                        mms = []
                        if s > 0:
                            for k in range(KD):
                                mms.append(
                                    (a_sc[:, k // 2, k % 2, ts(i, P)], mhat[:, k, :])
                                )
                        for j in range(i + 1):
                            mms.append(
                                (
                                    at_s[
                                        :,
                                        ab + A_OFF[j] + (i - j) * P : ab
                                        + A_OFF[j]
                                        + (i - j + 1) * P,
                                    ],
                                    vhat[:, s * NBLK + j, :],
                                )
                            )
                        for mi, (l_, r_) in enumerate(mms):
                            nc.tensor.matmul(
                                ps_y[:], l_, r_, start=(mi == 0), stop=(mi == len(mms) - 1)
                            )
                        rec = recp.tile([P, 1], f32, tag="rec")
                        nc.vector.reciprocal(rec[:], ps_y[:, DV : DV + 1])
                        y_sb = yp_.tile([P, DV], f32, tag="ysb")
                        nc.vector.tensor_scalar(y_sb[:], ps_y[:, 0:DV], rec[:], None, MUL)
                        nc.gpsimd.dma_start(y_out[ts(blk, P), :], y_sb[:])

                    # Mhat += K^T Vhat for this superchunk
                    for k in range(KD):
                        ps_d = dps.tile([P, DVE], f32, tag="d")
                        for i in range(NBLK):
                            nc.tensor.matmul(
                                ps_d[:],
                                kn_s[:, i, k // 2, (k % 2) * P : (k % 2 + 1) * P],
                                vhat[:, s * NBLK + i, :],
                                start=(i == 0),
                                stop=(i == NBLK - 1),
                            )
                        nc.vector.tensor_add(mhat[:, k, :], mhat[:, k, :], ps_d[:])

                pend = [stage_front(0), stage_front(1)]
                for s in range(NSC):
                    if s + 2 < NSC:
                        pend.append(stage_front(s + 2))
                    stage_back(pend.pop(0))

    nc.compile()
    return nc


def kernel(x, Wq, Wk, Wv):
    import ml_dtypes

    from concourse.bass_utils import run_bass_kernel_spmd

    x = np.ascontiguousarray(np.asarray(x, dtype=np.float32))
    Wq = np.asarray(Wq, dtype=np.float32)
    Wk = np.asarray(Wk, dtype=np.float32)
    Wv = np.asarray(Wv, dtype=np.float32)

    bf = ml_dtypes.bfloat16
    csT = np.ascontiguousarray(_cs_factors().T)           # [D, T]
    xT = np.ascontiguousarray(x.T).astype(bf)             # [D, T]

    in_maps = []
    for m in range(NCORE):
        sl = slice(m * DV, (m + 1) * DV)
        in_maps.append(
            {
                "xT": xT,
                "wqT": np.ascontiguousarray(Wq[sl, :].T).astype(bf),
                "wkT": np.ascontiguousarray(Wk[sl, :].T).astype(bf),
                "wvT": np.ascontiguousarray(Wv[sl, :].T).astype(bf),
                "csT": np.ascontiguousarray(csT[sl, :]),
            }
        )

    if "nc" not in _CACHE:
        _CACHE["nc"] = _build_nc()
    nc = _CACHE["nc"]

    trace = bool(int(os.environ.get("KERNEL_TRACE", "0")))
    res = run_bass_kernel_spmd(nc, in_maps, core_ids=list(range(NCORE)), trace=trace)
    _CACHE["last_result"] = res

    return np.concatenate([res.results[m]["y"] for m in range(NCORE)], axis=1)


# revision 25
# speedup vs baseline: 1.0879x; 1.0879x over previous
"""Trainium2 Bass kernel for nn_MemoryCell (causal linear attention memory cell).

Math: the reference's sequential scan
    mem += outer(k_t, v_t); zeta += k_t; y_t = (q_t @ mem) / (q_t . zeta)
is causal linear attention, computed as chunked attention over superchunks of
SC=512 steps:
    y = Q @ Mhat + tril(Q K^T) @ Vhat      (Vhat has an all-ones column
                                            which produces the normalizer)
    Mhat += K^T Vhat  per superchunk.

Sharding (8 cores, feature/tensor parallel): each core projects its 256-wide
slice of gated Q^T/K^T from full x, all-gathers them (bf16 wire), and computes
its 256-wide V/Mhat/y column slice locally. y is concatenated on the host.

v3 structure:
  - A = tril(K Q^T) is computed as per-core partials over the local 256
    features (no gather needed) and AllReduce-summed on the collective
    engine — instead of every core redoing the full-D contraction (8x
    redundant PE work in v1).
  - K is gathered pre-transposed (t-major) for the Mhat update: the local
    k slice is PE-transposed in phase 1 and the stacked [8*TH, DV] gather
    serves plain strided kn loads in phase 3 (v1/v2 used per-tile
    DMA_TRANSPOSE loads whose descriptor generation saturated ACT).
  - q (feature-major) and kT ride ONE AllGather per half (2 MB/rank).
  - phase 1 is split per half into [q/k projections + partial A -> fire
    AG+AR] then [V projections] so the collectives start ~70us in and
    overlap the remaining compute. Vhat stays resident in SBUF.
  - DMA queues separated: loads on sync (p1) / scalar (p3), stores on
    gpsimd SWDGE, y on sync — a v1 bounce write stuck behind phase-3
    loads delayed an AllGather by ~100us.

dtypes: x/W/q/k and A wire in bf16; all PSUM accumulation fp32.
"""

import os

import numpy as np

T, D = 4096, 2048
NCORE = 8
DV = D // NCORE          # 256: v-columns per core
DVE = DV + 2             # v-columns + ones column + pad (fp32r needs even N)
P = 128
KD = D // P              # 16 feature tiles
SC = 512                 # superchunk length (t)
NSC = T // SC            # 8
NBLK = SC // P           # 4 blocks per superchunk
NSCH = NSC // 2          # superchunks per half
TH = T // 2
A_OFF = [0, 512, 896, 1152]           # packed col offset of row j in an A tile
A_SPAN = [512, 384, 256, 128]         # row j covers t in [j*128, 512)
A_W = 1280                            # packed width per superchunk

_CACHE = {}


def _cs_factors():
    idx = np.arange(D // 2, dtype=np.float32)
    thetas = np.float32(10000.0) ** (np.float32(-2.0) * idx)
    pos = np.arange(T, dtype=np.float32)
    ang = pos[:, None] * thetas[None, :]
    cos = np.repeat(np.cos(ang), 2, axis=-1)
    sin = np.repeat(np.sin(ang), 2, axis=-1)
    return (cos + sin).astype(np.float32)


def _build_nc():
    import concourse.bacc as bacc
    import concourse.mybir as mybir
    import concourse.tile as tile
    from concourse.bass import ts
    from concourse.masks import make_identity, make_upper_triangular

    f32 = mybir.dt.float32
    bf16 = mybir.dt.bfloat16
    SIG = mybir.ActivationFunctionType.Sigmoid
    MUL = mybir.AluOpType.mult

    nc = bacc.Bacc(num_devices=NCORE)

    xT = nc.dram_tensor("xT", [D, T], bf16, kind="ExternalInput")
    wqT = nc.dram_tensor("wqT", [D, DV], bf16, kind="ExternalInput")
    wkT = nc.dram_tensor("wkT", [D, DV], bf16, kind="ExternalInput")
    wvT = nc.dram_tensor("wvT", [D, DV], bf16, kind="ExternalInput")
    csT = nc.dram_tensor("csT", [DV, T], f32, kind="ExternalInput")
    y_out = nc.dram_tensor("y", [T, DV], f32, kind="ExternalOutput")

    xTv = xT[:, :].rearrange("(k p) t -> p k t", p=P)     # [128, 16, T]
    wqv = wqT[:, :].rearrange("(k p) n -> p k n", p=P)    # [128, 16, 256]
    wkv = wkT[:, :].rearrange("(k p) n -> p k n", p=P)
    wvv = wvT[:, :].rearrange("(k p) n -> p k n", p=P)
    csv = csT[:, :].rearrange("(k p) t -> p k t", p=P)    # [128, 2, T]

    with tile.TileContext(nc) as tc:
        with (
            tc.tile_pool(name="const", bufs=1) as constp,
            tc.tile_pool(name="dram", bufs=1, space="DRAM") as dramp,
            tc.tile_pool(name="mhat", bufs=1) as mhatp,
            tc.tile_pool(name="vhat", bufs=1) as vhp,
        ):
            triu = constp.tile([P, P], f32)
            make_upper_triangular(nc, triu[:], val=1.0, diag=True)
            ident = constp.tile([P, P], bf16)
            make_identity(nc, ident[:])

            NQK = 2 * DV * TH  # flat bf16 elems per rank: q block + kT block
            qk_bounce = [dramp.tile([NQK], bf16, name=f"qk_bounce{h}") for h in range(2)]
            a_part = [dramp.tile([P, NSCH * A_W], bf16, name=f"a_part{h}") for h in range(2)]
            qk_gath = [
                dramp.tile([NCORE, NQK], bf16, addr_space="Shared", name=f"qk_gath{h}")
                for h in range(2)
            ]
            a_red = [
                dramp.tile([P, NSCH * A_W], bf16, addr_space="Shared", name=f"a_red{h}")
                for h in range(2)
            ]

            # bounce views: q block [DV, TH] feature-major; kT block [TH, DV]
            qb_v = [
                b[0:DV * TH].rearrange("(k p t) -> p k t", p=P, t=TH) for b in qk_bounce
            ]  # [128, 2, TH]
            ktb_v = [
                b[DV * TH : NQK].rearrange("(c b p n) -> p c b n", c=NSCH, b=NBLK, p=P)
                for b in qk_bounce
            ]  # [128(t), chunk, blk, 256]
            # gathered views
            qg_v = [
                g[:, 0:DV * TH].rearrange("m (k p t) -> p m k t", k=2, p=P)
                for g in qk_gath
            ]  # [128, 8, 2, TH] feature-major q; tile k_glob = m*2+k
            ktg_v = [
                g[:, DV * TH : NQK].rearrange("m (t n) -> t m n", n=DV)
                for g in qk_gath
            ]  # [TH(t), 8, 256] t-major k

            mhat = mhatp.tile([P, KD, DVE], bf16)         # [128, 16, 258]
            nc.vector.memset(mhat[:], 0.0)
            vhat = vhp.tile([P, NSC * NBLK, DVE], bf16)   # [128, 32, 258]
            nc.vector.memset(vhat[:, :, DV:DVE], 1.0)
            wv_sb = constp.tile([P, KD, DV], bf16)
            nc.sync.dma_start(wv_sb[:], wvv)

            def all_gather(src, dst):
                nc.gpsimd.collective_compute(
                    "AllGather",
                    mybir.AluOpType.bypass,
                    replica_groups=[list(range(NCORE))],
                    ins=[src.opt()],
                    outs=[dst.opt()],
                )

            def all_reduce(src, dst):
                nc.gpsimd.collective_compute(
                    "AllReduce",
                    mybir.AluOpType.add,
                    replica_groups=[list(range(NCORE))],
                    ins=[src.opt()],
                    outs=[dst.opt()],
                )

            # ---------------- Phase 1 ----------------
            with (
                tc.tile_pool(name="w", bufs=1) as wp,
                tc.tile_pool(name="xin", bufs=5) as xp,
                tc.tile_pool(name="csp", bufs=2) as csp,
                tc.tile_pool(name="qk", bufs=3) as qkp,
                tc.tile_pool(name="ktsb", bufs=2) as ktp,
                tc.tile_pool(name="asb", bufs=2) as asbp,
                tc.tile_pool(name="pj_ps", bufs=3, space="PSUM") as pjps,
                tc.tile_pool(name="px_ps", bufs=2, space="PSUM") as pxps,
                tc.tile_pool(name="pa_ps", bufs=2, space="PSUM") as paps,
            ):
                wq_sb = wp.tile([P, KD, DV], bf16)
                nc.sync.dma_start(wq_sb[:], wqv)
                wk_sb = wp.tile([P, KD, DV], bf16)
                nc.sync.dma_start(wk_sb[:], wkv)

                xts = {}

                def seg_a(h):
                    """q/k projections+gates, kT transpose, partial A, bounce writes."""
                    for ch in range(NSCH):
                        c = h * NSCH + ch
                        xt = xp.tile([P, KD, SC], bf16, tag="xt", name=f"xt{c}")
                        nc.sync.dma_start(xt[:], xTv[:, :, ts(c, SC)])
                        xts[c] = xt
                        cst = csp.tile([P, 2, SC], f32, tag="cst")
                        nc.sync.dma_start(cst[:], csv[:, :, ts(c, SC)])

                        gs = {}
                        for nm, w_sb in (("q", wq_sb), ("k", wk_sb)):
                            g = qkp.tile([P, 2, SC], bf16, tag=f"g{nm}")
                            for do in range(2):
                                ps = pjps.tile([P, SC], f32, tag="pj")
                                for k in range(KD):
                                    nc.tensor.matmul(
                                        ps[:],
                                        w_sb[:, k, ts(do, P)],
                                        xt[:, k, :],
                                        start=(k == 0),
                                        stop=(k == KD - 1),
                                    )
                                nc.vector.tensor_mul(g[:, do, :], ps[:], cst[:, do, :])
                                nc.scalar.activation(
                                    g[:, do, :], g[:, do, :], SIG, scale=1.0 / D
                                )
                            gs[nm] = g
                        # q bounce (feature-major)
                        nc.scalar.dma_start(qb_v[h][:, :, ts(ch, SC)], gs["q"][:])
                        # kT via PE transpose -> bounce (t-major)
                        kt_sb = ktp.tile([P, NBLK, DV], bf16, tag="kt")
                        for tb in range(NBLK):
                            for do in range(2):
                                pst = pxps.tile([P, DV], bf16, tag="px")
                                nc.tensor.transpose(
                                    pst[:, 0:P], gs["k"][:, do, ts(tb, P)], ident[:]
                                )
                                nc.vector.tensor_copy(
                                    kt_sb[:, tb, ts(do, P)], pst[:, 0:P]
                                )
                        nc.scalar.dma_start(ktb_v[h][:, ch, :, :], kt_sb[:])

                        # partial A rows for this superchunk (local features)
                        for j in range(NBLK):
                            span = A_SPAN[j]
                            ps_a = paps.tile([P, SC], f32, tag="pa")
                            for kt in range(2):
                                nc.tensor.matmul(
                                    ps_a[:, 0:span],
                                    gs["k"][:, kt, ts(j, P)],
                                    gs["q"][:, kt, j * P : SC],
                                    start=(kt == 0),
                                    stop=(kt == 1),
                                )
                            a_sb = asbp.tile([P, SC], bf16, tag="asb")
                            nc.vector.tensor_mul(a_sb[:, 0:P], ps_a[:, 0:P], triu[:])
                            if span > P:
                                nc.vector.tensor_copy(a_sb[:, P:span], ps_a[:, P:span])
                            nc.scalar.dma_start(
                                a_part[h][
                                    :, ch * A_W + A_OFF[j] : ch * A_W + A_OFF[j] + span
                                ],
                                a_sb[:, 0:span],
                            )

                def seg_b(h):
                    """V projections into resident vhat."""
                    for ch in range(NSCH):
                        c = h * NSCH + ch
                        xt = xts.pop(c)
                        for tt in range(NBLK):
                            psv = pxps.tile([P, DV], f32, tag="px")
                            for k in range(KD):
                                nc.tensor.matmul(
                                    psv[:],
                                    xt[:, k, ts(tt, P)],
                                    wv_sb[:, k, :],
                                    start=(k == 0),
                                    stop=(k == KD - 1),
                                )
                            nc.vector.tensor_copy(vhat[:, c * NBLK + tt, 0:DV], psv[:])

                seg_a(0)
                all_gather(qk_bounce[0], qk_gath[0])
                all_reduce(a_part[0], a_red[0])
                seg_b(0)
                seg_a(1)
                all_gather(qk_bounce[1], qk_gath[1])
                all_reduce(a_part[1], a_red[1])
                seg_b(1)

            # ---------------- Phase 3: chunked causal linear attention ----------------
            with (
                tc.tile_pool(name="asc", bufs=3) as ap_,
                tc.tile_pool(name="kn", bufs=3) as knp,
                tc.tile_pool(name="atsb", bufs=3) as atp,
                tc.tile_pool(name="ysb", bufs=6) as yp_,
                tc.tile_pool(name="rec", bufs=6) as recp,
                tc.tile_pool(name="y_ps", bufs=3, space="PSUM") as yps,
                tc.tile_pool(name="d_ps", bufs=3, space="PSUM") as dps,
            ):

                def stage_front(s):
                    """Loads for superchunk s."""
                    h, sh = divmod(s, NSCH)
                    a_sc = ap_.tile([P, NCORE, 2, SC], bf16, tag="a", name=f"a{s}")
                    for k2 in range(2):
                        nc.sync.dma_start(
                            a_sc[:, :, k2, :], qg_v[h][:, :, k2, ts(sh, SC)]
                        )
                    kn_s = knp.tile([P, NBLK, NCORE, DV], bf16, tag="kn", name=f"kn{s}")
                    for i in range(NBLK):
                        blk_h = sh * NBLK + i
                        nc.scalar.dma_start(
                            kn_s[:, i, :, :], ktg_v[h][ts(blk_h, P), :, :]
                        )
                    at_s = atp.tile([P, A_W], bf16, tag="at", name=f"at{s}")
                    nc.sync.dma_start(at_s[:], a_red[h][:, ts(sh, A_W)])
                    return (s, a_sc, kn_s, at_s)

                def stage_back(st):
                    """y blocks + Mhat update for a prepared superchunk."""
                    s, a_sc, kn_s, at_s = st
                    for i in range(NBLK):
                        blk = s * NBLK + i
                        ps_y = yps.tile([P, DVE], f32, tag="y")
                        mms = []
                        if s > 0:
                            for k in range(KD):
                                mms.append(
                                    (a_sc[:, k // 2, k % 2, ts(i, P)], mhat[:, k, :])
                                )
                        for j in range(i + 1):
                            mms.append(
                                (
                                    at_s[:, A_OFF[j] + (i - j) * P : A_OFF[j] + (i - j + 1) * P],
                                    vhat[:, s * NBLK + j, :],
                                )
                            )
                        for mi, (l_, r_) in enumerate(mms):
                            nc.tensor.matmul(
                                ps_y[:], l_, r_, start=(mi == 0), stop=(mi == len(mms) - 1)
                            )
                        rec = recp.tile([P, 1], f32, tag="rec")
                        nc.vector.reciprocal(rec[:], ps_y[:, DV : DV + 1])
                        y_sb = yp_.tile([P, DV], f32, tag="ysb")
                        nc.vector.tensor_scalar(y_sb[:], ps_y[:, 0:DV], rec[:], None, MUL)
                        nc.gpsimd.dma_start(y_out[ts(blk, P), :], y_sb[:])

                    # Mhat += K^T Vhat for this superchunk
                    for k in range(KD):
                        ps_d = dps.tile([P, DVE], f32, tag="d")
                        for i in range(NBLK):
                            nc.tensor.matmul(
                                ps_d[:],
                                kn_s[:, i, k // 2, (k % 2) * P : (k % 2 + 1) * P],
                                vhat[:, s * NBLK + i, :],
                                start=(i == 0),
                                stop=(i == NBLK - 1),
                            )
                        nc.vector.tensor_add(mhat[:, k, :], mhat[:, k, :], ps_d[:])

                pend = [stage_front(0), stage_front(1)]
                for s in range(NSC):
                    if s + 2 < NSC:
                        pend.append(stage_front(s + 2))
                    stage_back(pend.pop(0))

    nc.compile()
    return nc


def kernel(x, Wq, Wk, Wv):
    import ml_dtypes

    from concourse.bass_utils import run_bass_kernel_spmd

    x = np.ascontiguousarray(np.asarray(x, dtype=np.float32))
    Wq = np.asarray(Wq, dtype=np.float32)
    Wk = np.asarray(Wk, dtype=np.float32)
    Wv = np.asarray(Wv, dtype=np.float32)

    bf = ml_dtypes.bfloat16
    csT = np.ascontiguousarray(_cs_factors().T)           # [D, T]
    xT = np.ascontiguousarray(x.T).astype(bf)             # [D, T]

    in_maps = []
    for m in range(NCORE):
        sl = slice(m * DV, (m + 1) * DV)
        in_maps.append(
            {
                "xT": xT,
                "wqT": np.ascontiguousarray(Wq[sl, :].T).astype(bf),
                "wkT": np.ascontiguousarray(Wk[sl, :].T).astype(bf),
                "wvT": np.ascontiguousarray(Wv[sl, :].T).astype(bf),
                "csT": np.ascontiguousarray(csT[sl, :]),
            }
        )

    if "nc" not in _CACHE:
        _CACHE["nc"] = _build_nc()
    nc = _CACHE["nc"]

    trace = bool(int(os.environ.get("KERNEL_TRACE", "0")))
    res = run_bass_kernel_spmd(nc, in_maps, core_ids=list(range(NCORE)), trace=trace)
    _CACHE["last_result"] = res

    return np.concatenate([res.results[m]["y"] for m in range(NCORE)], axis=1)


# revision 26
# speedup vs baseline: 1.1959x; 1.0993x over previous
"""Trainium2 Bass kernel for nn_MemoryCell (causal linear attention memory cell).

Math: the reference's sequential scan
    mem += outer(k_t, v_t); zeta += k_t; y_t = (q_t @ mem) / (q_t . zeta)
is causal linear attention, computed as chunked attention over superchunks of
SC=512 steps:
    y = Q @ Mhat + tril(Q K^T) @ Vhat      (Vhat has an all-ones column
                                            which produces the normalizer)
    Mhat += K^T Vhat  per superchunk.

Sharding (8 cores, feature/tensor parallel): each core projects its 256-wide
slice of gated Q^T/K^T from full x, all-gathers them (bf16 wire), and computes
its 256-wide V/Mhat/y column slice locally. y is concatenated on the host.

v3 structure:
  - A = tril(K Q^T) is computed as per-core partials over the local 256
    features (no gather needed) and AllReduce-summed on the collective
    engine — instead of every core redoing the full-D contraction (8x
    redundant PE work in v1).
  - K is gathered pre-transposed (t-major) for the Mhat update: the local
    k slice is PE-transposed in phase 1 and the stacked [8*TH, DV] gather
    serves plain strided kn loads in phase 3 (v1/v2 used per-tile
    DMA_TRANSPOSE loads whose descriptor generation saturated ACT).
  - q (feature-major) and kT ride ONE AllGather per half (2 MB/rank).
  - phase 1 is split per half into [q/k projections + partial A -> fire
    AG+AR] then [V projections] so the collectives start ~70us in and
    overlap the remaining compute. Vhat stays resident in SBUF.
  - DMA queues separated: loads on sync (p1) / scalar (p3), stores on
    gpsimd SWDGE, y on sync — a v1 bounce write stuck behind phase-3
    loads delayed an AllGather by ~100us.

dtypes: x/W/q/k and A wire in bf16; all PSUM accumulation fp32.
"""

import os

import numpy as np

T, D = 4096, 2048
NCORE = 8
DV = D // NCORE          # 256: v-columns per core
DVE = DV + 2             # v-columns + ones column + pad (fp32r needs even N)
P = 128
KD = D // P              # 16 feature tiles
SC = 512                 # superchunk length (t)
NSC = T // SC            # 8
NBLK = SC // P           # 4 blocks per superchunk
NSCH = NSC // 2          # superchunks per half
TH = T // 2
A_OFF = [0, 512, 896, 1152]           # packed col offset of row j in an A tile
A_SPAN = [512, 384, 256, 128]         # row j covers t in [j*128, 512)
A_W = 1280                            # packed width per superchunk

_CACHE = {}


def _cs_factors():
    idx = np.arange(D // 2, dtype=np.float32)
    thetas = np.float32(10000.0) ** (np.float32(-2.0) * idx)
    pos = np.arange(T, dtype=np.float32)
    ang = pos[:, None] * thetas[None, :]
    cos = np.repeat(np.cos(ang), 2, axis=-1)
    sin = np.repeat(np.sin(ang), 2, axis=-1)
    return (cos + sin).astype(np.float32)


def _build_nc():
    import concourse.bacc as bacc
    import concourse.mybir as mybir
    import concourse.tile as tile
    from concourse.bass import ts
    from concourse.masks import make_identity, make_upper_triangular

    f32 = mybir.dt.float32
    bf16 = mybir.dt.bfloat16
    SIG = mybir.ActivationFunctionType.Sigmoid
    MUL = mybir.AluOpType.mult

    nc = bacc.Bacc(num_devices=NCORE)

    xT = nc.dram_tensor("xT", [D, T], bf16, kind="ExternalInput")
    wqT = nc.dram_tensor("wqT", [D, DV], bf16, kind="ExternalInput")
    wkT = nc.dram_tensor("wkT", [D, DV], bf16, kind="ExternalInput")
    wvT = nc.dram_tensor("wvT", [D, DV], bf16, kind="ExternalInput")
    csT = nc.dram_tensor("csT", [DV, T], f32, kind="ExternalInput")
    y_out = nc.dram_tensor("y", [T, DV], f32, kind="ExternalOutput")

    xTv = xT[:, :].rearrange("(k p) t -> p k t", p=P)     # [128, 16, T]
    wqv = wqT[:, :].rearrange("(k p) n -> p k n", p=P)    # [128, 16, 256]
    wkv = wkT[:, :].rearrange("(k p) n -> p k n", p=P)
    wvv = wvT[:, :].rearrange("(k p) n -> p k n", p=P)
    csv = csT[:, :].rearrange("(k p) t -> p k t", p=P)    # [128, 2, T]

    with tile.TileContext(nc) as tc:
        with (
            tc.tile_pool(name="const", bufs=1) as constp,
            tc.tile_pool(name="dram", bufs=1, space="DRAM") as dramp,
            tc.tile_pool(name="mhat", bufs=1) as mhatp,
            tc.tile_pool(name="vhat", bufs=1) as vhp,
        ):
            triu = constp.tile([P, P], f32)
            make_upper_triangular(nc, triu[:], val=1.0, diag=True)
            ident = constp.tile([P, P], bf16)
            make_identity(nc, ident[:])

            QE = DV * TH  # flat bf16 elems per rank per block (q or kT)
            q_bounce = [dramp.tile([QE], bf16, name=f"q_bounce{h}") for h in range(2)]
            kt_bounce = [dramp.tile([QE], bf16, name=f"kt_bounce{h}") for h in range(2)]
            a_part = [dramp.tile([P, NSCH * A_W], bf16, name=f"a_part{h}") for h in range(2)]
            q_gath = [
                dramp.tile([NCORE, QE], bf16, addr_space="Shared", name=f"q_gath{h}")
                for h in range(2)
            ]
            kt_gath = [
                dramp.tile([NCORE, QE], bf16, addr_space="Shared", name=f"kt_gath{h}")
                for h in range(2)
            ]
            a_red = [
                dramp.tile([P, NSCH * A_W], bf16, addr_space="Shared", name=f"a_red{h}")
                for h in range(2)
            ]

            # bounce views: q block [DV, TH] feature-major; kT block [TH, DV]
            qb_v = [
                b[0:QE].rearrange("(k p t) -> p k t", p=P, t=TH) for b in q_bounce
            ]  # [128, 2, TH]
            ktb_v = [
                b[0:QE].rearrange("(c b p n) -> p c b n", c=NSCH, b=NBLK, p=P)
                for b in kt_bounce
            ]  # [128(t), chunk, blk, 256]
            # gathered views
            qg_v = [
                g[:, :].rearrange("m (k p t) -> p m k t", k=2, p=P)
                for g in q_gath
            ]  # [128, 8, 2, TH] feature-major q; tile k_glob = m*2+k
            ktg_v = [
                g[:, :].rearrange("m (t n) -> t m n", n=DV)
                for g in kt_gath
            ]  # [TH(t), 8, 256] t-major k

            mhat = mhatp.tile([P, KD, DVE], bf16)         # [128, 16, 258]
            nc.vector.memset(mhat[:], 0.0)
            vhat = vhp.tile([P, NSC * NBLK, DVE], bf16)   # [128, 32, 258]
            nc.vector.memset(vhat[:, :, DV:DVE], 1.0)
            wv_sb = constp.tile([P, KD, DV], bf16)
            nc.sync.dma_start(wv_sb[:], wvv)

            def all_gather(src, dst):
                nc.gpsimd.collective_compute(
                    "AllGather",
                    mybir.AluOpType.bypass,
                    replica_groups=[list(range(NCORE))],
                    ins=[src.opt()],
                    outs=[dst.opt()],
                )

            def all_reduce(src, dst):
                nc.gpsimd.collective_compute(
                    "AllReduce",
                    mybir.AluOpType.add,
                    replica_groups=[list(range(NCORE))],
                    ins=[src.opt()],
                    outs=[dst.opt()],
                )

            # ---------------- Phase 1 ----------------
            with (
                tc.tile_pool(name="w", bufs=1) as wp,
                tc.tile_pool(name="xin", bufs=5) as xp,
                tc.tile_pool(name="csp", bufs=2) as csp,
                tc.tile_pool(name="qk", bufs=3) as qkp,
                tc.tile_pool(name="ktsb", bufs=2) as ktp,
                tc.tile_pool(name="asb", bufs=2) as asbp,
                tc.tile_pool(name="pj_ps", bufs=3, space="PSUM") as pjps,
                tc.tile_pool(name="px_ps", bufs=2, space="PSUM") as pxps,
                tc.tile_pool(name="pa_ps", bufs=2, space="PSUM") as paps,
            ):
                wq_sb = wp.tile([P, KD, DV], bf16)
                nc.sync.dma_start(wq_sb[:], wqv)
                wk_sb = wp.tile([P, KD, DV], bf16)
                nc.sync.dma_start(wk_sb[:], wkv)

                xts = {}

                def seg_a(h):
                    """q/k projections+gates, kT transpose, partial A, bounce writes."""
                    for ch in range(NSCH):
                        c = h * NSCH + ch
                        xt = xp.tile([P, KD, SC], bf16, tag="xt", name=f"xt{c}")
                        nc.sync.dma_start(xt[:], xTv[:, :, ts(c, SC)])
                        xts[c] = xt
                        cst = csp.tile([P, 2, SC], f32, tag="cst")
                        nc.sync.dma_start(cst[:], csv[:, :, ts(c, SC)])

                        gs = {}
                        for nm, w_sb in (("q", wq_sb), ("k", wk_sb)):
                            g = qkp.tile([P, 2, SC], bf16, tag=f"g{nm}")
                            for do in range(2):
                                ps = pjps.tile([P, SC], f32, tag="pj")
                                for k in range(KD):
                                    nc.tensor.matmul(
                                        ps[:],
                                        w_sb[:, k, ts(do, P)],
                                        xt[:, k, :],
                                        start=(k == 0),
                                        stop=(k == KD - 1),
                                    )
                                nc.vector.tensor_mul(g[:, do, :], ps[:], cst[:, do, :])
                                nc.scalar.activation(
                                    g[:, do, :], g[:, do, :], SIG, scale=1.0 / D
                                )
                            gs[nm] = g
                        # q bounce (feature-major)
                        nc.scalar.dma_start(qb_v[h][:, :, ts(ch, SC)], gs["q"][:])
                        # kT via PE transpose -> bounce (t-major)
                        kt_sb = ktp.tile([P, NBLK, DV], bf16, tag="kt")
                        for tb in range(NBLK):
                            for do in range(2):
                                pst = pxps.tile([P, DV], bf16, tag="px")
                                nc.tensor.transpose(
                                    pst[:, 0:P], gs["k"][:, do, ts(tb, P)], ident[:]
                                )
                                nc.vector.tensor_copy(
                                    kt_sb[:, tb, ts(do, P)], pst[:, 0:P]
                                )
                        nc.scalar.dma_start(ktb_v[h][:, ch, :, :], kt_sb[:])

                        # partial A rows for this superchunk (local features)
                        for j in range(NBLK):
                            span = A_SPAN[j]
                            ps_a = paps.tile([P, SC], f32, tag="pa")
                            for kt in range(2):
                                nc.tensor.matmul(
                                    ps_a[:, 0:span],
                                    gs["k"][:, kt, ts(j, P)],
                                    gs["q"][:, kt, j * P : SC],
                                    start=(kt == 0),
                                    stop=(kt == 1),
                                )
                            a_sb = asbp.tile([P, SC], bf16, tag="asb")
                            nc.vector.tensor_mul(a_sb[:, 0:P], ps_a[:, 0:P], triu[:])
                            if span > P:
                                nc.vector.tensor_copy(a_sb[:, P:span], ps_a[:, P:span])
                            nc.scalar.dma_start(
                                a_part[h][
                                    :, ch * A_W + A_OFF[j] : ch * A_W + A_OFF[j] + span
                                ],
                                a_sb[:, 0:span],
                            )

                def seg_b(h):
                    """V projections into resident vhat."""
                    for ch in range(NSCH):
                        c = h * NSCH + ch
                        xt = xts.pop(c)
                        for tt in range(NBLK):
                            psv = pxps.tile([P, DV], f32, tag="px")
                            for k in range(KD):
                                nc.tensor.matmul(
                                    psv[:],
                                    xt[:, k, ts(tt, P)],
                                    wv_sb[:, k, :],
                                    start=(k == 0),
                                    stop=(k == KD - 1),
                                )
                            nc.vector.tensor_copy(vhat[:, c * NBLK + tt, 0:DV], psv[:])

                seg_a(0)
                all_gather(q_bounce[0], q_gath[0])
                all_gather(kt_bounce[0], kt_gath[0])
                all_reduce(a_part[0], a_red[0])
                seg_b(0)
                seg_a(1)
                all_gather(q_bounce[1], q_gath[1])
                all_reduce(a_part[1], a_red[1])
                all_gather(kt_bounce[1], kt_gath[1])
                seg_b(1)

            # ---------------- Phase 3: chunked causal linear attention ----------------
            with (
                tc.tile_pool(name="asc", bufs=3) as ap_,
                tc.tile_pool(name="kn", bufs=3) as knp,
                tc.tile_pool(name="atsb", bufs=3) as atp,
                tc.tile_pool(name="ysb", bufs=6) as yp_,
                tc.tile_pool(name="rec", bufs=6) as recp,
                tc.tile_pool(name="y_ps", bufs=3, space="PSUM") as yps,
                tc.tile_pool(name="d_ps", bufs=3, space="PSUM") as dps,
            ):

                def stage_front(s):
                    """Loads for superchunk s."""
                    h, sh = divmod(s, NSCH)
                    a_sc = ap_.tile([P, NCORE, 2, SC], bf16, tag="a", name=f"a{s}")
                    for k2 in range(2):
                        nc.sync.dma_start(
                            a_sc[:, :, k2, :], qg_v[h][:, :, k2, ts(sh, SC)]
                        )
                    kn_s = knp.tile([P, NBLK, NCORE, DV], bf16, tag="kn", name=f"kn{s}")
                    for i in range(NBLK):
                        blk_h = sh * NBLK + i
                        nc.scalar.dma_start(
                            kn_s[:, i, :, :], ktg_v[h][ts(blk_h, P), :, :]
                        )
                    at_s = atp.tile([P, A_W], bf16, tag="at", name=f"at{s}")
                    nc.sync.dma_start(at_s[:], a_red[h][:, ts(sh, A_W)])
                    return (s, a_sc, kn_s, at_s)

                def stage_back(st):
                    """y blocks + Mhat update for a prepared superchunk."""
                    s, a_sc, kn_s, at_s = st
                    for i in range(NBLK):
                        blk = s * NBLK + i
                        ps_y = yps.tile([P, DVE], f32, tag="y")
                        mms = []
                        if s > 0:
                            for k in range(KD):
                                mms.append(
                                    (a_sc[:, k // 2, k % 2, ts(i, P)], mhat[:, k, :])
                                )
                        for j in range(i + 1):
                            mms.append(
                                (
                                    at_s[:, A_OFF[j] + (i - j) * P : A_OFF[j] + (i - j + 1) * P],
                                    vhat[:, s * NBLK + j, :],
                                )
                            )
                        for mi, (l_, r_) in enumerate(mms):
                            nc.tensor.matmul(
                                ps_y[:], l_, r_, start=(mi == 0), stop=(mi == len(mms) - 1)
                            )
                        rec = recp.tile([P, 1], f32, tag="rec")
                        nc.vector.reciprocal(rec[:], ps_y[:, DV : DV + 1])
                        y_sb = yp_.tile([P, DV], f32, tag="ysb")
                        nc.vector.tensor_scalar(y_sb[:], ps_y[:, 0:DV], rec[:], None, MUL)
                        nc.gpsimd.dma_start(y_out[ts(blk, P), :], y_sb[:])

                    # Mhat += K^T Vhat for this superchunk
                    for k in range(KD):
                        ps_d = dps.tile([P, DVE], f32, tag="d")
                        for i in range(NBLK):
                            nc.tensor.matmul(
                                ps_d[:],
                                kn_s[:, i, k // 2, (k % 2) * P : (k % 2 + 1) * P],
                                vhat[:, s * NBLK + i, :],
                                start=(i == 0),
                                stop=(i == NBLK - 1),
                            )
                        nc.vector.tensor_add(mhat[:, k, :], mhat[:, k, :], ps_d[:])

                pend = [stage_front(0), stage_front(1)]
                for s in range(NSC):
                    if s + 2 < NSC:
                        pend.append(stage_front(s + 2))
                    stage_back(pend.pop(0))

    nc.compile()
    return nc


def kernel(x, Wq, Wk, Wv):
    import ml_dtypes

    from concourse.bass_utils import run_bass_kernel_spmd

    x = np.ascontiguousarray(np.asarray(x, dtype=np.float32))
    Wq = np.asarray(Wq, dtype=np.float32)
    Wk = np.asarray(Wk, dtype=np.float32)
    Wv = np.asarray(Wv, dtype=np.float32)

    bf = ml_dtypes.bfloat16
    csT = np.ascontiguousarray(_cs_factors().T)           # [D, T]
    xT = np.ascontiguousarray(x.T).astype(bf)             # [D, T]

    in_maps = []
    for m in range(NCORE):
        sl = slice(m * DV, (m + 1) * DV)
        in_maps.append(
            {
                "xT": xT,
                "wqT": np.ascontiguousarray(Wq[sl, :].T).astype(bf),
                "wkT": np.ascontiguousarray(Wk[sl, :].T).astype(bf),
                "wvT": np.ascontiguousarray(Wv[sl, :].T).astype(bf),
                "csT": np.ascontiguousarray(csT[sl, :]),
            }
        )

    if "nc" not in _CACHE:
        _CACHE["nc"] = _build_nc()
    nc = _CACHE["nc"]

    trace = bool(int(os.environ.get("KERNEL_TRACE", "0")))
    res = run_bass_kernel_spmd(nc, in_maps, core_ids=list(range(NCORE)), trace=trace)
    _CACHE["last_result"] = res

    return np.concatenate([res.results[m]["y"] for m in range(NCORE)], axis=1)
